# revision 3
# baseline (speedup 1.0000x reference)
"""Bass/Trainium2 kernel for nn_Attention_6983616824415.

Reference computation (per batch b):
    K = elu(Wk @ H[b] + bk).T          # [S, D]
    V = elu(Wv @ H[b] + bv).T          # [S, D]
    E = Q @ K.T                        # [L, S]
    A = softmax(E, axis=-1)
    C = A @ V                          # [L, D]
    returns (C, A)

Sharding: 8 cores = (batch b = core//2) x (L-half = core%2). K/V are computed
per-batch on the two cores owning that batch; Q rows are split in half and
zero-padded to LPAD=4480.

Per-core on-chip layout: everything is organized so the softmax reduction
(over S) lands on the PE contraction axis:
    KT[d, s] = elu(Wk @ H + bk)        # d on partitions
    V[s, d]  = elu(H.T @ Wv.T + bv)    # s on partitions
    ET[s, l] = KT.T @ QT               # per 512-wide l-block
    P[s, l]  = exp(ET)                 # no max-subtraction: |E| <~ 2
    Craw[l,d] = P.T @ V, sums[l] = P.T @ 1   (fused, shared stationary operand)
    C[l, d]  = Craw * (1/sums)         # per-partition scale
    A^T[s,l] = P * bcast(1/sums)       # row broadcast via K=1 matmul
A is written to DRAM transposed ([S, L] per core) and untransposed on the
host during unsharding.  All matmuls run in bf16 (fp32 PE matmul is 4x
slower); accumulation is fp32 in PSUM.
"""

import os
import sys

import numpy as np

for _p in ("/root/.axon_site/_ro/trn_rl_repo", "/opt/trn_rl_repo"):
    if os.path.isdir(_p) and _p not in sys.path:
        sys.path.append(_p)

B, D, S, L = 4, 512, 2048, 8921
N_CORES = 8
LPAD = 4480  # padded per-core label count (L split 4461/4460)
L_SPLIT = 4461


def build_body(nc, tile, mybir, bass, dims, io):
    """Emit one full per-core kernel body inside an open TileContext.

    dims: dict with D, S, LP (padded labels)
    io: dict of DRAM tensor handles: qt, h, wkt, wvt, bk, bv, at, c
    """
    from concourse.masks import make_identity

    f32 = mybir.dt.float32
    bf16 = mybir.dt.bfloat16
    Exp = mybir.ActivationFunctionType.Exp
    Copy = mybir.ActivationFunctionType.Copy
    op = mybir.AluOpType

    Dd, Ss, LP = dims["D"], dims["S"], dims["LP"]
    nd = Dd // 128          # d tiles (contraction for KT/V/ET)
    ns = Ss // 128          # s tiles
    s_blocks = [(s0, min(512, Ss - s0)) for s0 in range(0, Ss, 512)]
    l_blocks = [(l0, min(512, LP - l0)) for l0 in range(0, LP, 512)]

    qt, h, wkt, wvt, bk_d, bv_d, at_d, c_d = (
        io["qt"], io["h"], io["wkt"], io["wvt"], io["bk"], io["bv"],
        io["at"], io["c"],
    )

    tc = io["tc"]
    ctx = io["ctx"]

    const = ctx.enter_context(tc.tile_pool(name="const", bufs=1))
    wpool = ctx.enter_context(tc.tile_pool(name="wpool", bufs=1))
    kvpool = ctx.enter_context(tc.tile_pool(name="kvpool", bufs=1))
    tmp = ctx.enter_context(tc.tile_pool(name="tmp", bufs=3))
    exp_pool = ctx.enter_context(tc.tile_pool(name="exp", bufs=2 * ns))
    outp = ctx.enter_context(tc.tile_pool(name="outp", bufs=6))
    rec = ctx.enter_context(tc.tile_pool(name="rec", bufs=2))
    p_mm = ctx.enter_context(tc.tile_pool(name="p_mm", bufs=2, space="PSUM"))
    p_cr = ctx.enter_context(tc.tile_pool(name="p_cr", bufs=2, space="PSUM"))
    p_sm = ctx.enter_context(tc.tile_pool(name="p_sm", bufs=2, space="PSUM"))
    p_tr = ctx.enter_context(tc.tile_pool(name="p_tr", bufs=1, space="PSUM"))
    p_bc = ctx.enter_context(tc.tile_pool(name="p_bc", bufs=1, space="PSUM"))

    # ---- constants -------------------------------------------------------
    ident = const.tile([128, 128], f32)
    make_identity(nc, ident)
    ones_col = const.tile([128, 1], bf16)
    nc.vector.memset(ones_col, 1.0)
    ones_row = const.tile([1, 128], f32)
    nc.vector.memset(ones_row, 1.0)

    # biases: bk as [128, nd] (column t = bk[128*t:128*(t+1)]), bv broadcast
    bk_col = const.tile([128, nd], f32)
    nc.gpsimd.dma_start(bk_col, bk_d.ap().rearrange("(t p) -> p t", p=128))
    bkm1_col = const.tile([128, nd], f32)
    nc.vector.tensor_scalar_sub(bkm1_col, bk_col, 1.0)
    bv_bc = const.tile([128, Dd], f32)
    nc.gpsimd.dma_start(
        bv_bc, bass.AP(tensor=bv_d, offset=0, ap=[[0, 128], [1, Dd]])
    )

    # ---- weights / inputs -----------------------------------------------
    wkt_sb = []
    wvt_sb = []
    h_sb = []
    qt_sb = []
    for t in range(nd):
        wk_t = wpool.tile([128, Dd], bf16, name=f"wk_{t}")
        nc.sync.dma_start(wk_t, wkt.ap()[t * 128:(t + 1) * 128, :])
        wkt_sb.append(wk_t)
        wv_t = wpool.tile([128, Dd], bf16, name=f"wv_{t}")
        nc.sync.dma_start(wv_t, wvt.ap()[t * 128:(t + 1) * 128, :])
        wvt_sb.append(wv_t)
        h_t = wpool.tile([128, Ss], bf16, name=f"h_{t}")
        nc.sync.dma_start(h_t, h.ap()[t * 128:(t + 1) * 128, :])
        h_sb.append(h_t)
        q_t = wpool.tile([128, LP], bf16, name=f"q_{t}")
        nc.sync.dma_start(q_t, qt.ap()[t * 128:(t + 1) * 128, :])
        qt_sb.append(q_t)

    # ---- KT = elu(Wk @ H + bk): [d, s], d on partitions ------------------
    kt_sb = []
    for o in range(nd):
        kt_t = kvpool.tile([128, Ss], bf16, name=f"kt_{o}")
        kt_sb.append(kt_t)
    for o in range(nd):
        for (s0, sb) in s_blocks:
            ps = p_mm.tile([128, 512], f32, tag="mm")
            for dt_i in range(nd):
                nc.tensor.matmul(
                    ps[:, :sb],
                    wkt_sb[dt_i][:, o * 128:(o + 1) * 128],
                    h_sb[dt_i][:, s0:s0 + sb],
                    start=(dt_i == 0),
                    stop=(dt_i == nd - 1),
                )
            # elu(z) with z = ps + bk[o]:
            #   m  = min(z, 0);  e = exp(m);  r1 = max(z, 0) - 1 = max(z-1+bk, -1)
            #   kt = r1 + e
            m_t = tmp.tile([128, 512], f32, tag="m")
            nc.vector.tensor_scalar(
                m_t[:, :sb], ps[:, :sb], bk_col[:, o:o + 1], 0.0,
                op0=op.add, op1=op.min,
            )
            e_t = tmp.tile([128, 512], f32, tag="e")
            nc.scalar.activation(e_t[:, :sb], m_t[:, :sb], Exp)
            r_t = tmp.tile([128, 512], f32, tag="r")
            nc.vector.tensor_scalar(
                r_t[:, :sb], ps[:, :sb], bkm1_col[:, o:o + 1], -1.0,
                op0=op.add, op1=op.max,
            )
            nc.vector.tensor_tensor(
                kt_sb[o][:, s0:s0 + sb], r_t[:, :sb], e_t[:, :sb], op=op.add
            )

    # ---- V = elu(H.T @ Wv.T + bv): [s, d], s on partitions ---------------
    v_sb = []
    for st in range(ns):
        v_t = kvpool.tile([128, Dd], bf16, name=f"v_{st}")
        v_sb.append(v_t)
    for st in range(ns):
        ps = p_mm.tile([128, 512], f32, tag="mm")
        for dt_i in range(nd):
            nc.tensor.matmul(
                ps[:, :Dd],
                h_sb[dt_i][:, st * 128:(st + 1) * 128],
                wvt_sb[dt_i],
                start=(dt_i == 0),
                stop=(dt_i == nd - 1),
            )
        # z = ps + bv (bias varies along free dim -> tensor_tensor)
        z_t = tmp.tile([128, 512], f32, tag="z")
        nc.vector.tensor_tensor(z_t[:, :Dd], ps[:, :Dd], bv_bc, op=op.add)
        m_t = tmp.tile([128, 512], f32, tag="m")
        nc.vector.tensor_scalar_min(m_t[:, :Dd], z_t[:, :Dd], 0.0)
        e_t = tmp.tile([128, 512], f32, tag="e")
        nc.scalar.activation(e_t[:, :Dd], m_t[:, :Dd], Exp)
        r_t = tmp.tile([128, 512], f32, tag="r")
        nc.vector.tensor_scalar(
            r_t[:, :Dd], z_t[:, :Dd], 0.0, -1.0, op0=op.max, op1=op.add
        )
        nc.vector.tensor_tensor(v_sb[st], r_t[:, :Dd], e_t[:, :Dd], op=op.add)

    # ---- main loop over l-blocks ----------------------------------------
    for (l0, lb) in l_blocks:
        nlt = lb // 128
        # ET = KT.T @ QT -> exp
        exp_sb = []
        for st in range(ns):
            ps = p_mm.tile([128, 512], f32, tag="mm")
            for dt_i in range(nd):
                nc.tensor.matmul(
                    ps[:, :lb],
                    kt_sb[dt_i][:, st * 128:(st + 1) * 128],
                    qt_sb[dt_i][:, l0:l0 + lb],
                    start=(dt_i == 0),
                    stop=(dt_i == nd - 1),
                )
            p_t = exp_pool.tile([128, 512], bf16, tag="exp", name=f"p_{l0}_{st}")
            nc.scalar.activation(p_t[:, :lb], ps[:, :lb], Exp)
            exp_sb.append(p_t)

        # Craw = P.T @ V (+ fused sums = P.T @ 1), then C = Craw / sums
        recip_all = rec.tile([128, 4], f32, tag="recip")
        for lt in range(nlt):
            ps_c = p_cr.tile([128, Dd], f32, tag="craw")
            ps_s = p_sm.tile([128, 1], f32, tag="sums")
            for st in range(ns):
                w_ap = exp_sb[st][:, lt * 128:(lt + 1) * 128]
                nc.tensor.matmul(
                    ps_c, w_ap, v_sb[st],
                    start=(st == 0), stop=(st == ns - 1),
                    skip_group_check=True,
                )
                nc.tensor.matmul(
                    ps_s, w_ap, ones_col,
                    start=(st == 0), stop=(st == ns - 1),
                    skip_group_check=True,
                )
            nc.vector.reciprocal(recip_all[:, lt:lt + 1], ps_s)
            c_t = outp.tile([128, Dd], bf16, tag="c_out")
            nc.vector.tensor_scalar_mul(c_t, ps_c, recip_all[:, lt:lt + 1])
            nc.sync.dma_start(c_d.ap()[l0 + lt * 128:l0 + (lt + 1) * 128, :], c_t)

        # broadcast recip over partitions: transpose -> row -> K=1 matmul
        ps_t = p_tr.tile([4, 128], f32, tag="tr")
        nc.tensor.transpose(ps_t[:nlt, :], recip_all[:, :nlt], ident)
        row4 = rec.tile([4, 128], f32, tag="row4")
        nc.scalar.activation(row4[:nlt, :], ps_t[:nlt, :], Copy)
        row_sb = rec.tile([1, 512], f32, tag="row")
        nc.sync.dma_start(row_sb[0:1, :lb], row4[:nlt, :])
        ps_b = p_bc.tile([128, 512], f32, tag="bc")
        nc.tensor.matmul(ps_b[:, :lb], ones_row, row_sb[0:1, :lb])
        recip_bc = rec.tile([128, 512], bf16, tag="rbc")
        nc.scalar.activation(recip_bc[:, :lb], ps_b[:, :lb], Copy)

        # A^T tiles: P * recip_bc -> DRAM (transposed layout [S, LP])
        for st in range(ns):
            a_t = outp.tile([128, 512], bf16, tag="a_out")
            nc.vector.tensor_tensor(
                a_t[:, :lb], exp_sb[st][:, :lb], recip_bc[:, :lb], op=op.mult
            )
            nc.sync.dma_start(
                at_d.ap()[st * 128:(st + 1) * 128, l0:l0 + lb], a_t[:, :lb]
            )


def build_nc(dims, num_devices=N_CORES, repeats=1):
    """Build and compile the full Bass program. Returns nc."""
    from contextlib import ExitStack

    import concourse.bass as bass
    from concourse import bacc, mybir, tile

    f32 = mybir.dt.float32
    bf16 = mybir.dt.bfloat16
    Dd, Ss, LP = dims["D"], dims["S"], dims["LP"]

    nc = bacc.Bacc("TRN2", debug=False, enable_asserts=False,
                   num_devices=num_devices)
    io = {
        "qt": nc.dram_tensor("qt", [Dd, LP], bf16, kind="ExternalInput"),
        "h": nc.dram_tensor("h", [Dd, Ss], bf16, kind="ExternalInput"),
        "wkt": nc.dram_tensor("wkt", [Dd, Dd], bf16, kind="ExternalInput"),
        "wvt": nc.dram_tensor("wvt", [Dd, Dd], bf16, kind="ExternalInput"),
        "bk": nc.dram_tensor("bk", [Dd], f32, kind="ExternalInput"),
        "bv": nc.dram_tensor("bv", [Dd], f32, kind="ExternalInput"),
        "at": nc.dram_tensor("at", [Ss, LP], bf16, kind="ExternalOutput"),
        "c": nc.dram_tensor("c", [LP, Dd], bf16, kind="ExternalOutput"),
    }
    with ExitStack() as ctx:
        tc = ctx.enter_context(tile.TileContext(nc))
        io["tc"] = tc
        for _ in range(repeats):
            with ExitStack() as body_ctx:
                io["ctx"] = body_ctx
                build_body(nc, tile, mybir, bass, dims, io)
    nc.compile()
    return nc


_CACHE = {}


def _get_nc():
    key = "full"
    if key not in _CACHE:
        _CACHE[key] = build_nc({"D": D, "S": S, "LP": LPAD})
    return _CACHE[key]


def make_in_maps(H, Wk, bk, Wv, bv, Q):
    import ml_dtypes

    bf16 = ml_dtypes.bfloat16
    wkt = np.ascontiguousarray(Wk.T).astype(bf16)
    wvt = np.ascontiguousarray(Wv.T).astype(bf16)
    bk = np.ascontiguousarray(bk).astype(np.float32)
    bv = np.ascontiguousarray(bv).astype(np.float32)
    QT = np.ascontiguousarray(Q.T)  # [D, L] f32
    qt_halves = []
    for (l0, l1) in ((0, L_SPLIT), (L_SPLIT, L)):
        qp = np.zeros((D, LPAD), dtype=bf16)
        qp[:, : l1 - l0] = QT[:, l0:l1].astype(bf16)
        qt_halves.append(qp)
    in_maps = []
    for c in range(N_CORES):
        b, half = c // 2, c % 2
        in_maps.append({
            "qt": qt_halves[half],
            "h": np.ascontiguousarray(H[b]).astype(bf16),
            "wkt": wkt,
            "wvt": wvt,
            "bk": bk,
            "bv": bv,
        })
    return in_maps


def kernel(H, Wk, bk, Wv, bv, Q):
    from concourse import bass_utils

    nc = _get_nc()
    in_maps = make_in_maps(H, Wk, bk, Wv, bv, Q)
    res = bass_utils.run_bass_kernel_spmd(
        nc, in_maps, core_ids=list(range(N_CORES))
    )
    C = np.empty((B, L, D), dtype=np.float32)
    A = np.empty((B, L, S), dtype=np.float32)
    for c in range(N_CORES):
        b, half = c // 2, c % 2
        l0, l1 = ((0, L_SPLIT), (L_SPLIT, L))[half]
        n = l1 - l0
        C[b, l0:l1] = res.results[c]["c"][:n].astype(np.float32)
        A[b, l0:l1] = res.results[c]["at"][:, :n].T.astype(np.float32)
    return C, A


# revision 21
# speedup vs baseline: 431.7139x; 431.7139x over previous
"""Bass/Trainium2 kernel for nn_Attention_6983616824415.

Reference computation (per batch b):
    K = elu(Wk @ H[b] + bk).T          # [S, D]
    V = elu(Wv @ H[b] + bv).T          # [S, D]
    E = Q @ K.T                        # [L, S]
    A = softmax(E, axis=-1)
    C = A @ V                          # [L, D]
    returns (C, A)

Sharding: 8 cores = (batch b = core//2) x (L-half = core%2). K/V are computed
per-batch on the two cores owning that batch; Q rows are split in half and
zero-padded to LPAD=4480.

Per-core on-chip layout: everything is organized so the softmax reduction
(over S) lands on the PE contraction axis:
    KT[d, s] = elu(Wk @ H + bk)        # d on partitions
    V[s, d]  = elu(H.T @ Wv.T + bv)    # s on partitions
    ET[s, l] = KT.T @ QT               # per 512-wide l-block
    P[s, l]  = exp(ET)                 # no max-subtraction: |E| <~ 2
    Craw[l,d] = P.T @ V, sums[l] = P.T @ 1   (fused, shared stationary operand)
    C[l, d]  = Craw * (1/sums)         # per-partition scale
    A^T[s,l] = P * bcast(1/sums)       # row broadcast via K=1 matmul
A is written to DRAM transposed ([S, L] per core) and untransposed on the
host during unsharding.  All matmuls run in bf16 (fp32 PE matmul is 4x
slower); accumulation is fp32 in PSUM.
"""

import os
import sys

import numpy as np

for _p in ("/root/.axon_site/_ro/trn_rl_repo", "/opt/trn_rl_repo"):
    if os.path.isdir(_p) and _p not in sys.path:
        sys.path.append(_p)

B, D, S, L = 4, 512, 2048, 8921
N_CORES = 8
LPAD = 4480  # padded per-core label count (L split 4461/4460)
L_SPLIT = 4461


def build_body(nc, tile, mybir, bass, dims, io):
    """Emit one full per-core kernel body inside an open TileContext.

    dims: dict with D, S, LP (padded labels)
    io: dict of DRAM tensor handles: qt, h, wkt, wvt, bk, bv, at, c
    """
    from concourse.masks import make_identity

    f32 = mybir.dt.float32
    bf16 = mybir.dt.bfloat16
    Exp = mybir.ActivationFunctionType.Exp
    Copy = mybir.ActivationFunctionType.Copy
    op = mybir.AluOpType

    Dd, Ss, LP = dims["D"], dims["S"], dims["LP"]
    nd = Dd // 128          # d tiles (contraction for KT/V/ET)
    ns = Ss // 128          # s tiles
    s_blocks = [(s0, min(512, Ss - s0)) for s0 in range(0, Ss, 512)]
    l_blocks = [(l0, min(512, LP - l0)) for l0 in range(0, LP, 512)]

    qt, h, wkt, wvt, bk_d, bv_d, at_d, c_d = (
        io["qt"], io["h"], io["wkt"], io["wvt"], io["bk"], io["bv"],
        io["at"], io["c"],
    )

    tc = io["tc"]
    ctx = io["ctx"]

    const = ctx.enter_context(tc.tile_pool(name="const", bufs=1))
    wpool = ctx.enter_context(tc.tile_pool(name="wpool", bufs=1))
    kvpool = ctx.enter_context(tc.tile_pool(name="kvpool", bufs=1))
    tmp = ctx.enter_context(tc.tile_pool(name="tmp", bufs=3))
    exp_pool = ctx.enter_context(tc.tile_pool(name="exp", bufs=2 * ns))
    outp = ctx.enter_context(tc.tile_pool(name="outp", bufs=6))
    rec = ctx.enter_context(tc.tile_pool(name="rec", bufs=2))
    p_mm = ctx.enter_context(tc.tile_pool(name="p_mm", bufs=3, space="PSUM"))
    p_crA = ctx.enter_context(tc.tile_pool(name="p_crA", bufs=2, space="PSUM"))
    p_crB = ctx.enter_context(tc.tile_pool(name="p_crB", bufs=2, space="PSUM"))
    # transpose + broadcast psums share one bank (their uses are serial links
    # of the same per-block chain)
    p_tr = ctx.enter_context(tc.tile_pool(name="p_tr", bufs=1, space="PSUM"))
    p_bc = p_tr

    # ---- constants -------------------------------------------------------
    ident = const.tile([128, 128], f32)
    make_identity(nc, ident)
    ones_row = const.tile([1, 128], bf16)
    nc.vector.memset(ones_row, 1.0)

    # biases: bk as [128, nd] (column t = bk[128*t:128*(t+1)]); bv as a
    # [1, D] bf16 row — its add is folded into the V matmul as a K=1
    # accumulation (ones_row.T @ bv_row).
    bk_col = const.tile([128, nd], f32)
    nc.gpsimd.dma_start(bk_col, bk_d.ap().rearrange("(t p) -> p t", p=128))
    bv_f32 = const.tile([1, Dd], f32)
    nc.gpsimd.dma_start(bv_f32, bv_d.ap().rearrange("(o d) -> o d", o=1))
    bv_row = const.tile([1, Dd], bf16)
    nc.vector.tensor_copy(bv_row, bv_f32)

    # ---- weights / inputs (K-path tensors first so KT matmuls start early,
    # qt last since it is only needed by the main loop) -------------------
    wkt_sb = []
    wvt_sb = []
    h_sb = []
    qt_sb = []
    for t in range(nd):
        wk_t = wpool.tile([128, Dd], bf16, name=f"wk_{t}")
        nc.sync.dma_start(wk_t, wkt.ap()[t * 128:(t + 1) * 128, :])
        wkt_sb.append(wk_t)
        h_t = wpool.tile([128, Ss], bf16, name=f"h_{t}")
        for (s0, sb) in s_blocks:
            nc.sync.dma_start(
                h_t[:, s0:s0 + sb],
                h.ap()[t * 128:(t + 1) * 128, s0:s0 + sb],
            )
        h_sb.append(h_t)
    for t in range(nd):
        wv_t = wpool.tile([128, Dd], bf16, name=f"wv_{t}")
        nc.sync.dma_start(wv_t, wvt.ap()[t * 128:(t + 1) * 128, :])
        wvt_sb.append(wv_t)
    for t in range(nd):
        q_t = wpool.tile([128, LP], bf16, name=f"q_{t}")
        nc.sync.dma_start(q_t, qt.ap()[t * 128:(t + 1) * 128, :])
        qt_sb.append(q_t)

    # ---- KT = elu(Wk @ H + bk): [d, s], d on partitions ------------------
    kt_sb = []
    for o in range(nd):
        kt_t = kvpool.tile([128, Ss], bf16, name=f"kt_{o}")
        kt_sb.append(kt_t)
    for o in range(nd):
        for (s0, sb) in s_blocks:
            ps = p_mm.tile([128, 512], f32, tag="mm")
            for dt_i in range(nd):
                nc.tensor.matmul(
                    ps[:, :sb],
                    wkt_sb[dt_i][:, o * 128:(o + 1) * 128],
                    h_sb[dt_i][:, s0:s0 + sb],
                    start=(dt_i == 0),
                    stop=(dt_i == nd - 1),
                )
            # elu(z) = max(z, min(exp(z), 1) - 1), z = ps + bk[o]
            # (exact: z<=0 <=> exp(z)<=1, and exp(z)-1 >= z everywhere)
            e_t = tmp.tile([128, 512], f32, tag="e")
            nc.scalar.activation(e_t[:, :sb], ps[:, :sb], Exp,
                                 bias=bk_col[:, o:o + 1])
            t_t = tmp.tile([128, 512], f32, tag="t")
            nc.vector.tensor_scalar(
                t_t[:, :sb], e_t[:, :sb], 1.0, -1.0, op0=op.min, op1=op.add
            )
            nc.vector.scalar_tensor_tensor(
                kt_sb[o][:, s0:s0 + sb], ps[:, :sb], bk_col[:, o:o + 1],
                t_t[:, :sb], op0=op.add, op1=op.max,
            )

    # ---- V = elu(H.T @ Wv.T + bv): [s, d], s on partitions ---------------
    # v tiles carry an extra ones-column (col Dd) so the softmax sums fold
    # into the second Craw matmul for free.
    v_sb = []
    for st in range(ns):
        v_t = kvpool.tile([128, Dd + 1], bf16, name=f"v_{st}")
        v_sb.append(v_t)
    for st in range(ns):
        # V psum borrows the craw-A slots (idle during the KV phase) so the
        # KT and V matmul pipelines don't serialize on p_mm's two banks.
        ps = p_crA.tile([128, 512], f32, tag="crawA")
        for dt_i in range(nd):
            nc.tensor.matmul(
                ps[:, :Dd],
                h_sb[dt_i][:, st * 128:(st + 1) * 128],
                wvt_sb[dt_i],
                start=(dt_i == 0),
                stop=False,
            )
        # + bv via K=1 accumulation: ones_row.T @ bv_row
        nc.tensor.matmul(ps[:, :Dd], ones_row, bv_row, start=False, stop=True)
        # elu(z) = max(z, min(exp(z),1) - 1), z already includes the bias
        e_t = tmp.tile([128, 512], f32, tag="e")
        nc.scalar.activation(e_t[:, :Dd], ps[:, :Dd], Exp)
        t_t = tmp.tile([128, 512], f32, tag="t")
        nc.vector.tensor_scalar(
            t_t[:, :Dd], e_t[:, :Dd], 1.0, -1.0, op0=op.min, op1=op.add
        )
        nc.vector.scalar_tensor_tensor(
            v_sb[st][:, :Dd], ps[:, :Dd], 0.0, t_t[:, :Dd],
            op0=op.add, op1=op.max,
        )
        nc.gpsimd.memset(v_sb[st][:, Dd:Dd + 1], 1.0)

    # ---- main loop over l-blocks ----------------------------------------
    for (l0, lb) in l_blocks:
        nlt = lb // 128
        # ET = KT.T @ QT -> exp
        exp_sb = []
        for st in range(ns):
            ps = p_mm.tile([128, 512], f32, tag="mm")
            for dt_i in range(nd):
                nc.tensor.matmul(
                    ps[:, :lb],
                    kt_sb[dt_i][:, st * 128:(st + 1) * 128],
                    qt_sb[dt_i][:, l0:l0 + lb],
                    start=(dt_i == 0),
                    stop=(dt_i == nd - 1),
                )
            p_t = exp_pool.tile([128, 512], bf16, tag="exp", name=f"p_{l0}_{st}")
            nc.scalar.activation(p_t[:, :lb], ps[:, :lb], Exp)
            exp_sb.append(p_t)

        # Craw = P.T @ [V | 1] split as N=256 + N=257 (sums ride in the last
        # column of the second matmul), then C = Craw / sums
        hD = Dd // 2
        recip_all = rec.tile([128, 4], f32, tag="recip")
        for lt in range(nlt):
            ps_a = p_crA.tile([128, hD], f32, tag="crawA")
            ps_b = p_crB.tile([128, hD + 1], f32, tag="crawB")
            for st in range(ns):
                w_ap = exp_sb[st][:, lt * 128:(lt + 1) * 128]
                nc.tensor.matmul(
                    ps_a, w_ap, v_sb[st][:, :hD],
                    start=(st == 0), stop=(st == ns - 1),
                    skip_group_check=True,
                )
                nc.tensor.matmul(
                    ps_b, w_ap, v_sb[st][:, hD:Dd + 1],
                    start=(st == 0), stop=(st == ns - 1),
                    skip_group_check=True,
                )
            nc.vector.reciprocal(recip_all[:, lt:lt + 1], ps_b[:, hD:hD + 1])
            c_t = outp.tile([128, Dd], bf16, tag="c_out")
            nc.vector.tensor_scalar_mul(
                c_t[:, :hD], ps_a, recip_all[:, lt:lt + 1]
            )
            nc.vector.tensor_scalar_mul(
                c_t[:, hD:Dd], ps_b[:, :hD], recip_all[:, lt:lt + 1]
            )
            nc.sync.dma_start(c_d.ap()[l0 + lt * 128:l0 + (lt + 1) * 128, :], c_t)

        # broadcast recip over partitions: transpose -> row -> K=1 matmul
        ps_t = p_tr.tile([4, 128], f32, tag="trbc")
        nc.tensor.transpose(ps_t[:nlt, :], recip_all[:, :nlt], ident)
        row4 = rec.tile([4, 128], bf16, tag="row4")
        nc.scalar.activation(row4[:nlt, :], ps_t[:nlt, :], Copy)
        row_sb = rec.tile([1, 512], bf16, tag="row")
        nc.sync.dma_start(row_sb[0:1, :lb], row4[:nlt, :])
        ps_b = p_bc.tile([128, 512], f32, tag="trbc")
        nc.tensor.matmul(ps_b[:, :lb], ones_row, row_sb[0:1, :lb])
        recip_bc = rec.tile([128, 512], bf16, tag="rbc")
        nc.scalar.activation(recip_bc[:, :lb], ps_b[:, :lb], Copy)

        # A^T tiles: P * recip_bc -> DRAM (transposed layout [S, LP])
        for st in range(ns):
            a_t = outp.tile([128, 512], bf16, tag="a_out")
            nc.vector.tensor_tensor(
                a_t[:, :lb], exp_sb[st][:, :lb], recip_bc[:, :lb], op=op.mult
            )
            nc.sync.dma_start(
                at_d.ap()[st * 128:(st + 1) * 128, l0:l0 + lb], a_t[:, :lb]
            )


def build_nc(dims, num_devices=N_CORES, repeats=1):
    """Build and compile the full Bass program. Returns nc."""
    from contextlib import ExitStack

    import concourse.bass as bass
    from concourse import bacc, mybir, tile

    f32 = mybir.dt.float32
    bf16 = mybir.dt.bfloat16
    Dd, Ss, LP = dims["D"], dims["S"], dims["LP"]

    nc = bacc.Bacc("TRN2", debug=False, enable_asserts=False,
                   num_devices=num_devices)
    io = {
        "qt": nc.dram_tensor("qt", [Dd, LP], bf16, kind="ExternalInput"),
        "h": nc.dram_tensor("h", [Dd, Ss], bf16, kind="ExternalInput"),
        "wkt": nc.dram_tensor("wkt", [Dd, Dd], bf16, kind="ExternalInput"),
        "wvt": nc.dram_tensor("wvt", [Dd, Dd], bf16, kind="ExternalInput"),
        "bk": nc.dram_tensor("bk", [Dd], f32, kind="ExternalInput"),
        "bv": nc.dram_tensor("bv", [Dd], f32, kind="ExternalInput"),
        "at": nc.dram_tensor("at", [Ss, LP], bf16, kind="ExternalOutput"),
        "c": nc.dram_tensor("c", [LP, Dd], bf16, kind="ExternalOutput"),
    }
    with ExitStack() as ctx:
        tc = ctx.enter_context(tile.TileContext(nc))
        io["tc"] = tc
        for _ in range(repeats):
            with ExitStack() as body_ctx:
                io["ctx"] = body_ctx
                build_body(nc, tile, mybir, bass, dims, io)
    nc.compile()
    return nc


_CACHE = {}


def _get_nc():
    key = "full"
    if key not in _CACHE:
        _CACHE[key] = build_nc({"D": D, "S": S, "LP": LPAD})
    return _CACHE[key]


def make_in_maps(H, Wk, bk, Wv, bv, Q):
    import ml_dtypes

    bf16 = ml_dtypes.bfloat16
    wkt = np.ascontiguousarray(Wk.T).astype(bf16)
    wvt = np.ascontiguousarray(Wv.T).astype(bf16)
    bk = np.ascontiguousarray(bk).astype(np.float32)
    bv = np.ascontiguousarray(bv).astype(np.float32)
    QT = np.ascontiguousarray(Q.T)  # [D, L] f32
    qt_halves = []
    for (l0, l1) in ((0, L_SPLIT), (L_SPLIT, L)):
        qp = np.zeros((D, LPAD), dtype=bf16)
        qp[:, : l1 - l0] = QT[:, l0:l1].astype(bf16)
        qt_halves.append(qp)
    in_maps = []
    for c in range(N_CORES):
        b, half = c // 2, c % 2
        in_maps.append({
            "qt": qt_halves[half],
            "h": np.ascontiguousarray(H[b]).astype(bf16),
            "wkt": wkt,
            "wvt": wvt,
            "bk": bk,
            "bv": bv,
        })
    return in_maps


def kernel(H, Wk, bk, Wv, bv, Q):
    from concourse import bass_utils

    nc = _get_nc()
    in_maps = make_in_maps(H, Wk, bk, Wv, bv, Q)
    res = bass_utils.run_bass_kernel_spmd(
        nc, in_maps, core_ids=list(range(N_CORES))
    )
    C = np.empty((B, L, D), dtype=np.float32)
    A = np.empty((B, L, S), dtype=np.float32)
    for c in range(N_CORES):
        b, half = c // 2, c % 2
        l0, l1 = ((0, L_SPLIT), (L_SPLIT, L))[half]
        n = l1 - l0
        C[b, l0:l1] = res.results[c]["c"][:n].astype(np.float32)
        A[b, l0:l1] = res.results[c]["at"][:, :n].T.astype(np.float32)
    return C, A
